# revision 1
# baseline (speedup 1.0000x reference)
"""Causal multi-head attention (B=4, S=2048, D=1024, H=16) on 8 Trainium2 NeuronCores.

Sharding: core c handles batch c//2 and head-group c%2 (8 of 16 heads).
Each core computes qkv projection (f32r matmuls), causal attention
(scores f32r, softmax via ACT exp, PV in bf16 with embedded row-sum
columns), and its 8 heads' slice of the output projection; the host sums
the two half-head partials per batch.

Self-contained: hardcodes shapes; imports concourse from the container's
trn_rl_repo. kernel(**inputs) takes full inputs, returns full output.
"""
import sys

for _p in ("/opt/trn_rl_repo", "/root/.axon_site/_ro/trn_rl_repo"):
    if _p not in sys.path:
        sys.path.append(_p)

import numpy as np

import concourse.bass as bass
import concourse.mybir as mybir
import concourse.tile as tile
from concourse import bacc
from concourse.masks import make_identity

B, S, D, H = 4, 2048, 1024, 16
HD = D // H            # 64
NHL = 8                # heads per core
QB = 1024              # attention q-block
NKC = S // 128         # 16 k-chunks per sequence
dt = mybir.dt
AF = mybir.ActivationFunctionType
P = 128


def build_nc(dbg=False, repeat=1, phases=3):
    nc = bacc.Bacc("TRN2", target_bir_lowering=False, debug=False)

    xs = nc.dram_tensor("xs", [S, D], dt.float32, kind="ExternalInput")
    wqk = nc.dram_tensor("wqk", [P, 8, 8, P], dt.float32, kind="ExternalInput")
    wv = nc.dram_tensor("wv", [P, 8, 4, P], dt.float32, kind="ExternalInput")
    wpj = nc.dram_tensor("wpj", [P, 4, D], dt.float32, kind="ExternalInput")
    out = nc.dram_tensor("out", [S, D], dt.float32, kind="ExternalOutput")
    if dbg:
        d_qt = nc.dram_tensor("d_qt", [P, 4, S], dt.float32, kind="ExternalOutput")
        d_kt = nc.dram_tensor("d_kt", [P, 4, S], dt.float32, kind="ExternalOutput")
        d_v2 = nc.dram_tensor("d_v2", [P, NHL, NKC, 96], dt.bfloat16, kind="ExternalOutput")
        d_yt = nc.dram_tensor("d_yt", [P, 4, S], dt.float32, kind="ExternalOutput")
        d_pv = nc.dram_tensor("d_pv", [P, QB], dt.float32, kind="ExternalOutput")
        d_pt = nc.dram_tensor("d_pt", [P, QB], dt.bfloat16, kind="ExternalOutput")

    from contextlib import ExitStack
    with tile.TileContext(nc) as tc, ExitStack() as _rep:
        if repeat > 1:
            _rep.enter_context(tc.For_i(0, repeat, 1))
        with tc.tile_pool(name="persist", bufs=1) as pp:

            ident = pp.tile([P, P], dt.float32, tag="ident")
            make_identity(nc, ident[:])
            # causal mask tile: 0 where f>=p else -1e30
            maskT = pp.tile([P, P], dt.float32, tag="maskT")
            nc.gpsimd.memset(maskT[:], 0.0)
            nc.gpsimd.affine_select(
                out=maskT[:], in_=maskT[:],
                compare_op=mybir.AluOpType.is_ge, fill=-1e30,
                base=0, pattern=[[1, P]], channel_multiplier=-1)

            QT = pp.tile([P, 4, S], dt.float32r, tag="QT")  # [hd(2-head pair), pair, s]
            KT = pp.tile([P, 4, S], dt.float32r, tag="KT")
            V2 = pp.tile([P, NHL, NKC, 96], dt.bfloat16, tag="V2")  # [k, head, kc, 64 V | 32 ones]
            nc.gpsimd.memset(V2[:, :, :, 64:96], 1.0)
            yT = pp.tile([P, 4, S], dt.float32r, tag="yT")  # [y-dim pair, pair, s]

            # ---------------- Phase Q: x^T, then QKV projections ----------------
            with tc.tile_pool(name="tq1", bufs=1) as tq1, \
                 tc.tile_pool(name="tq", bufs=2) as tq, \
                 tc.tile_pool(name="psA", bufs=4, space="PSUM") as psA:
                for sb in range(4):            # s-blocks of 512
                    xT = tq1.tile([P, 8, 512], dt.float32r, tag="xT")  # [d, dc, s-in-block]
                    for sc in range(4):        # 128-row chunks
                        xn = tq.tile([P, D], dt.float32, tag="xn")
                        nc.sync.dma_start(xn[:], xs[(sb * 4 + sc) * P:(sb * 4 + sc + 1) * P, :])
                        for g in range(2):     # transpose 4 d-chunks per psum tile
                            ptr = psA.tile([P, 512], dt.float32, tag="pmm")
                            for j in range(4):
                                dc = g * 4 + j
                                nc.tensor.transpose(ptr[:, j * P:(j + 1) * P],
                                                    xn[:, dc * P:(dc + 1) * P], ident[:])
                            nc.vector.tensor_copy(
                                xT[:, g * 4:(g + 1) * 4, sc * P:(sc + 1) * P],
                                ptr[:].rearrange("p (j f) -> p j f", j=4))
                    for ch in range(8):        # 4 q-pairs then 4 k-pairs
                        wqkf = tq.tile([P, 8, P], dt.float32, tag="wqkf")
                        nc.sync.dma_start(wqkf[:], wqk[:, :, ch, :])
                        wqkc = tq.tile([P, 8, P], dt.float32r, tag="wqkc")
                        nc.vector.tensor_copy(wqkc[:], wqkf[:])
                        psq = psA.tile([P, 512], dt.float32, tag="pmm")
                        for dc in range(8):
                            nc.tensor.matmul(psq[:], wqkc[:, dc, :], xT[:, dc, :],
                                             start=(dc == 0), stop=(dc == 7))
                        if ch < 4:
                            nc.vector.tensor_copy(QT[:, ch, sb * 512:(sb + 1) * 512], psq[:])
                        else:
                            nc.vector.tensor_copy(KT[:, ch - 4, sb * 512:(sb + 1) * 512], psq[:])
                    for vc in range(4):        # v-pairs -> VT -> transpose -> V natural
                        wvf = tq.tile([P, 8, P], dt.float32, tag="wqkf")
                        nc.sync.dma_start(wvf[:], wv[:, :, vc, :])
                        wvc = tq.tile([P, 8, P], dt.float32r, tag="wqkc")
                        nc.vector.tensor_copy(wvc[:], wvf[:])
                        psv_ = psA.tile([P, 512], dt.float32, tag="pmm")
                        for dc in range(8):
                            nc.tensor.matmul(psv_[:], wvc[:, dc, :], xT[:, dc, :],
                                             start=(dc == 0), stop=(dc == 7))
                        vt = tq.tile([P, 512], dt.float32, tag="vt")
                        nc.vector.tensor_copy(vt[:], psv_[:])
                        for sc in range(4):
                            kc = sb * 4 + sc
                            ptv = psA.tile([P, 512], dt.float32, tag="pmm")
                            nc.tensor.transpose(ptv[:, 0:P], vt[:, sc * P:(sc + 1) * P], ident[:])
                            nc.vector.tensor_copy(V2[:, 2 * vc, kc, 0:64], ptv[:, 0:64])
                            nc.vector.tensor_copy(V2[:, 2 * vc + 1, kc, 0:64], ptv[:, 64:P])

            wpj_r = pp.tile([P, 4, D], dt.float32r, tag="wpj_r")
            with tc.tile_pool(name="wstage", bufs=1) as ws:
                wpjf = ws.tile([P, 4, D], dt.float32, tag="wpjf")
                nc.sync.dma_start(wpjf[:], wpj[:])
                nc.vector.tensor_copy(wpj_r[:], wpjf[:])

            # ---------------- Phase A: causal attention ----------------
            with tc.tile_pool(name="ta", bufs=2) as ta, \
                 tc.tile_pool(name="tpt", bufs=4) as tpt, \
                 tc.tile_pool(name="psS", bufs=2, space="PSUM") as psS, \
                 tc.tile_pool(name="psV", bufs=2, space="PSUM") as psV:
              for h in range(NHL if phases >= 2 else 0):  
                pr = h // 2
                half = slice(0, 64) if h % 2 == 0 else slice(64, P)
                for qb in range(2):
                    nkc = (qb + 1) * 8
                    pv_ps = psV.tile([P, QB], dt.float32, tag="pv")
                    pend = None  # (kc, pT tile, qlo)

                    def emit_pv(kc, pT_t, qlo):
                        q0 = qlo
                        while q0 < QB:
                            q1 = min((q0 // 512 + 1) * 512, QB)  # stay within one PSUM bank
                            nc.tensor.matmul(pv_ps[0:96, q0:q1],
                                             V2[:, h, kc, :], pT_t[:, q0:q1],
                                             start=(kc == 0), stop=(kc == nkc - 1),
                                             skip_group_check=True)
                            q0 = q1

                    for kc in range(nkc):
                        qlo = max(0, kc * P - qb * QB)
                        sc_ps = psS.tile([P, QB], dt.float32, tag="sc")
                        q0 = qlo
                        while q0 < QB:
                            q1 = min((q0 // 512 + 1) * 512, QB)  # stay within one PSUM bank
                            nc.tensor.matmul(sc_ps[:, q0:q1],
                                             KT[half, pr, kc * P:(kc + 1) * P],
                                             QT[half, pr, qb * QB + q0:qb * QB + q1],
                                             start=True, stop=True)
                            q0 = q1
                        if kc * P >= qb * QB:  # diagonal chunk: -1e30 on k>q corner
                            nc.vector.tensor_tensor(sc_ps[:, qlo:qlo + P],
                                                    sc_ps[:, qlo:qlo + P], maskT[:],
                                                    mybir.AluOpType.add)
                        pT_t = tpt.tile([P, QB], dt.bfloat16, tag="pT")
                        nc.scalar.activation(pT_t[:, qlo:QB], sc_ps[:, qlo:QB],
                                             AF.Exp, scale=0.125)
                        if pend is not None:
                            emit_pv(*pend)
                        pend = (kc, pT_t, qlo)
                    emit_pv(*pend)
                    if dbg and h == 0 and qb == 0:
                        dpv = ta.tile([P, QB], dt.float32, tag="dpv")
                        nc.vector.tensor_copy(dpv[:], pv_ps[:])
                        nc.sync.dma_start(d_pv[:], dpv[:])
                        nc.sync.dma_start(d_pt[:], pend[1][:])

                    # normalization: r = exp(-ln(sums)); sums dup on rows 64:96
                    tln = ta.tile([P, QB], dt.float32, tag="tln")
                    nc.scalar.activation(tln[64:96, :], pv_ps[64:96, :], AF.Ln)
                    trc = ta.tile([P, QB], dt.float32, tag="trc")
                    nc.scalar.activation(trc[64:96, :], tln[64:96, :], AF.Exp, scale=-1.0)
                    rsh = ta.tile([64, QB], dt.float32, tag="rsh")
                    nc.sync.dma_start(rsh[0:32, :], trc[64:96, :])
                    nc.sync.dma_start(rsh[32:64, :], trc[64:96, :])
                    if h % 2 == 0:
                        nc.vector.tensor_tensor(yT[0:64, pr, qb * QB:(qb + 1) * QB],
                                                pv_ps[0:64, :], rsh[:],
                                                mybir.AluOpType.mult)
                    else:
                        ytmp = ta.tile([64, QB], dt.float32r, tag="ytmp")
                        nc.vector.tensor_tensor(ytmp[:], pv_ps[0:64, :], rsh[:],
                                                mybir.AluOpType.mult)
                        nc.sync.dma_start(yT[64:P, pr, qb * QB:(qb + 1) * QB], ytmp[:])

            if dbg:
                nc.sync.dma_start(d_qt[:], QT[:].bitcast(dt.float32))
                nc.sync.dma_start(d_kt[:], KT[:].bitcast(dt.float32))
                nc.sync.dma_start(d_v2[:], V2[:])
                nc.sync.dma_start(d_yt[:], yT[:].bitcast(dt.float32))

            # ---------------- Phase P: output projection ----------------
            if phases < 3:
                # keep results live: dump QT/KT/V2/yT slices into out
                nc.sync.dma_start(out[0:P, 0:D], QT[:, 0, 0:D].bitcast(dt.float32).unsqueeze(1))
                nc.sync.dma_start(out[P:2 * P, 0:D], KT[:, 1, 0:D].bitcast(dt.float32).unsqueeze(1))
                if phases >= 2:
                    nc.sync.dma_start(out[2 * P:3 * P, 0:D], yT[:, 2, 0:D].bitcast(dt.float32).unsqueeze(1))
                so0 = pp.tile([P, D], dt.float32, tag="so0")
                nc.vector.tensor_copy(so0[:], V2[:, 0, :, :].rearrange("p a b -> p (a b)")[:, 0:D])
                nc.sync.dma_start(out[3 * P:4 * P, 0:D], so0[:])
            with tc.tile_pool(name="tp", bufs=4) as tp, \
                 tc.tile_pool(name="psP", bufs=6, space="PSUM") as psP:
                for sc in range(16 if phases >= 3 else 0):
                    for oc in range(2):
                        pps = psP.tile([P, 512], dt.float32, tag="pp")
                        for pc in range(4):
                            nc.tensor.matmul(pps[:], yT[:, pc, sc * P:(sc + 1) * P],
                                             wpj_r[:, pc, oc * 512:(oc + 1) * 512],
                                             start=(pc == 0), stop=(pc == 3))
                        so = tp.tile([P, 512], dt.float32, tag="so")
                        nc.vector.tensor_copy(so[:], pps[:])
                        nc.sync.dma_start(out[sc * P:(sc + 1) * P, oc * 512:(oc + 1) * 512], so[:])

    nc.compile()
    return nc


def prepare_inputs(x, Wqkv, Wproj):
    """Pack per-core inputs. Core c: batch c//2, heads (c%2)*8 .. +8."""
    x = np.asarray(x, dtype=np.float32)
    Wqkv = np.asarray(Wqkv, dtype=np.float32)
    Wproj = np.asarray(Wproj, dtype=np.float32)
    in_maps = []
    for c in range(8):
        b, g = c // 2, c % 2
        hg = g * NHL
        wqk = np.empty((P, 8, 8, P), dtype=np.float32)
        wv = np.empty((P, 8, 4, P), dtype=np.float32)
        # Wqkv rows d = dc*128 + p
        Wq = Wqkv[:, :D].reshape(8, P, H, HD)       # [dc, p, head, hd]
        Wk = Wqkv[:, D:2 * D].reshape(8, P, H, HD)
        Wv_ = Wqkv[:, 2 * D:].reshape(8, P, H, HD)
        for ch in range(4):
            wqk[:, :, ch, 0:64] = Wq[:, :, hg + 2 * ch, :].transpose(1, 0, 2)
            wqk[:, :, ch, 64:P] = Wq[:, :, hg + 2 * ch + 1, :].transpose(1, 0, 2)
            wqk[:, :, ch + 4, 0:64] = Wk[:, :, hg + 2 * ch, :].transpose(1, 0, 2)
            wqk[:, :, ch + 4, 64:P] = Wk[:, :, hg + 2 * ch + 1, :].transpose(1, 0, 2)
            wv[:, :, ch, 0:64] = Wv_[:, :, hg + 2 * ch, :].transpose(1, 0, 2)
            wv[:, :, ch, 64:P] = Wv_[:, :, hg + 2 * ch + 1, :].transpose(1, 0, 2)
        wpj = np.empty((P, 4, D), dtype=np.float32)
        for pc in range(4):
            wpj[0:64, pc, :] = Wproj[HD * (hg + 2 * pc):HD * (hg + 2 * pc) + HD, :]
            wpj[64:P, pc, :] = Wproj[HD * (hg + 2 * pc + 1):HD * (hg + 2 * pc + 1) + HD, :]
        in_maps.append({
            "xs": np.ascontiguousarray(x[b]),
            "wqk": wqk, "wv": wv, "wpj": wpj,
        })
    return in_maps


def combine_outputs(results):
    out = np.empty((B, S, D), dtype=np.float32)
    for b in range(B):
        out[b] = results[2 * b]["out"] + results[2 * b + 1]["out"]
    return out


_NC_CACHE = None


def get_nc():
    global _NC_CACHE
    if _NC_CACHE is None:
        _NC_CACHE = build_nc()
    return _NC_CACHE


def kernel(x, Wqkv, Wproj):
    from concourse.bass_utils import run_bass_kernel_spmd
    nc = get_nc()
    in_maps = prepare_inputs(x, Wqkv, Wproj)
    res = run_bass_kernel_spmd(nc, in_maps, core_ids=list(range(8)))
    return combine_outputs(res.results)


if __name__ == "__main__":
    rng = np.random.default_rng(0)
    x = rng.standard_normal((B, S, D), dtype=np.float32)
    Wqkv = (rng.standard_normal((D, 3 * D), dtype=np.float32) / np.sqrt(D)).astype(np.float32)
    Wproj = (rng.standard_normal((D, D), dtype=np.float32) / np.sqrt(D)).astype(np.float32)
    y = kernel(x, Wqkv, Wproj)
    print("ok", y.shape, float(np.abs(y).max()))



# revision 3
# speedup vs baseline: 1.7221x; 1.7221x over previous
"""Causal MHA (B=4, S=2048, D=1024, H=16) on 8 TRN2 cores — v2.

Core c: batch c//2, head-group c%2 (8 of 16 heads). All-bf16 data path
(fp32 PSUM accum), DMA-engine x-transpose, natural-V projection, DVE
reciprocal softmax denominator, GPSIMD causal corner zeroing, and
phase-interleaved emission (QKV half1 inside qb0 attention, proj half0
inside qb1 attention).
"""
import sys

for _p in ("/opt/trn_rl_repo", "/root/.axon_site/_ro/trn_rl_repo"):
    if _p not in sys.path:
        sys.path.append(_p)

import numpy as np
from ml_dtypes import bfloat16

import concourse.bass as bass
import concourse.mybir as mybir
import concourse.tile as tile
from concourse import bacc

B, S, D, H = 4, 2048, 1024, 16
HD = D // H            # 64
NHL = 8                # heads per core
QB = 1024              # attention q-block
NKC = S // 128         # 16 k-chunks
dt = mybir.dt
AF = mybir.ActivationFunctionType
ALU = mybir.AluOpType
P = 128


def build_nc(repeat=1):
    nc = bacc.Bacc("TRN2", target_bir_lowering=False, debug=False)

    xs16 = nc.dram_tensor("xs16", [S, D], dt.bfloat16, kind="ExternalInput")
    wqk = nc.dram_tensor("wqk", [P, 8, 8, P], dt.bfloat16, kind="ExternalInput")
    wv = nc.dram_tensor("wv", [P, 8, 512], dt.bfloat16, kind="ExternalInput")
    wpj = nc.dram_tensor("wpj", [P, 4, D], dt.bfloat16, kind="ExternalInput")
    out = nc.dram_tensor("out", [S, D], dt.bfloat16, kind="ExternalOutput")

    from contextlib import ExitStack
    with tile.TileContext(nc) as tc, ExitStack() as _rep:
        if repeat > 1:
            _rep.enter_context(tc.For_i(0, repeat, 1))
        with tc.tile_pool(name="persist", bufs=1) as pp, \
             tc.tile_pool(name="xt", bufs=2) as xtp, \
             tc.tile_pool(name="st", bufs=4) as stp, \
             tc.tile_pool(name="pt", bufs=4) as ptp, \
             tc.tile_pool(name="rs", bufs=4) as rsp, \
             tc.tile_pool(name="psB", bufs=3, space="PSUM") as psB, \
             tc.tile_pool(name="psV", bufs=1, space="PSUM") as psV:

            QT = pp.tile([P, 4, S], dt.bfloat16, tag="QT")   # [hd-in-pair, pair, s]
            KT = pp.tile([P, 4, S], dt.bfloat16, tag="KT")
            # [k, head, kc, col]: 64 V cols then 32 ones (row-sum trick)
            V2 = pp.tile([P, NHL, NKC, 96], dt.bfloat16, tag="V2")
            yT = pp.tile([P, 4, S], dt.bfloat16, tag="yT")
            nc.gpsimd.memset(V2[:, :, :, 64:96], 1.0)

            wqk16 = pp.tile([P, 8, 8, P], dt.bfloat16, tag="wqk16")
            wv16 = pp.tile([P, 8, 512], dt.bfloat16, tag="wv16")
            wpj_r = pp.tile([P, 4, D], dt.bfloat16, tag="wpj_r")
            xT0 = xtp.tile([P, 8, 1024], dt.bfloat16, tag="xT")
            xT1 = xtp.tile([P, 8, 1024], dt.bfloat16, tag="xT")
            xT = [xT0, xT1]
            nc.sync.dma_start(wqk16[:, :, 0:2, :], wqk[:, :, 0:2, :])
            nc.sync.dma_start_transpose(xT[0][:], xs16[0:1024, :])
            nc.sync.dma_start(wqk16[:, :, 2:8, :], wqk[:, :, 2:8, :])
            nc.sync.dma_start(wv16[:], wv[:])
            nc.sync.dma_start(wpj_r[:], wpj[:])
            nc.sync.dma_start_transpose(xT[1][:], xs16[1024:2048, :])

            def qk_unit(half, sb, chp):
                """ch pair (2chp, 2chp+1) over s-block sb (512) of half."""
                ps = psB.tile([P, QB], dt.float32, tag="ps")
                for j in range(2):
                    ch = 2 * chp + j
                    for dc in range(8):
                        nc.tensor.matmul(
                            ps[:, j * 512:(j + 1) * 512],
                            wqk16[:, dc, ch, :],
                            xT[half][:, dc, sb * 512:(sb + 1) * 512],
                            start=(dc == 0), stop=(dc == 7))
                soff = half * 1024 + sb * 512
                dst = QT if chp < 2 else KT
                pr0 = (2 * chp) % 4
                nc.vector.tensor_copy(
                    dst[:, pr0:pr0 + 2, soff:soff + 512],
                    ps[:].rearrange("p (j f) -> p j f", j=2))

            def v_unit(half, scp):
                """s-chunk pair (2scp, 2scp+1) within half -> V natural."""
                ps = psB.tile([P, QB], dt.float32, tag="ps")
                for j in range(2):
                    sl = (2 * scp + j) * 128
                    for dc in range(8):
                        nc.tensor.matmul(
                            ps[:, j * 512:(j + 1) * 512],
                            xT[half][:, dc, sl:sl + 128],
                            wv16[:, dc, :],
                            start=(dc == 0), stop=(dc == 7))
                kc0 = half * 8 + 2 * scp
                src = ps[:].rearrange("p (j h f) -> p h j f", j=2, h=8)
                nc.vector.tensor_copy(V2[:, :, kc0:kc0 + 2, 0:64], src)

            def proj_unit(sc):
                ps = psB.tile([P, QB], dt.float32, tag="ps")
                for oc in range(2):
                    for pc in range(4):
                        nc.tensor.matmul(
                            ps[:, oc * 512:(oc + 1) * 512],
                            yT[:, pc, sc * P:(sc + 1) * P],
                            wpj_r[:, pc, oc * 512:(oc + 1) * 512],
                            start=(pc == 0), stop=(pc == 3))
                so = stp.tile([P, QB], dt.bfloat16, tag="so")
                nc.vector.tensor_copy(so[:], ps[:])
                nc.sync.dma_start(out[sc * P:(sc + 1) * P, :], so[:])

            def attn(h, qb, inject=None):
                pr, parity = h // 2, h % 2
                half = slice(0, 64) if parity == 0 else slice(64, P)
                nkc = (qb + 1) * 8
                base = qb * QB
                lastA = 3 if qb == 0 else 11   # last kc touching q < 512
                pv = psV.tile([P, QB], dt.float32, tag="pv")

                def pv_out(a, b):
                    return pv[0:96, a:b]

                def emit_pv(kc, pT_t, qlo, poff):
                    segs = []
                    if qlo < 512:
                        segs.append((qlo, 512, kc == lastA))
                    segs.append((max(qlo, 512), QB, kc == nkc - 1))
                    for a, b, sp in segs:
                        nc.tensor.matmul(
                            pv_out(a, b), V2[:, h, kc, :],
                            pT_t[:, poff + a - qlo:poff + b - qlo],
                            start=(kc == 0), stop=sp,
                            skip_group_check=True)

                # chunk groups sharing one PSUM tile + one exp: the last four
                # (narrow, diagonal) chunks are packed in pairs
                groups = []
                for kc in range(nkc - 4):
                    qlo = max(0, kc * P - base)
                    groups.append([(kc, qlo, qlo)])
                for k0 in (nkc - 4, nkc - 2):
                    g, poff = [], 0
                    for kc in (k0, k0 + 1):
                        qlo = kc * P - base
                        g.append((kc, qlo, poff))
                        poff += QB - qlo
                    groups.append(g)

                pend = []
                for gi, g in enumerate(groups):
                    ps = psB.tile([P, QB], dt.float32, tag="ps")
                    lo = min(poff for _, _, poff in g)
                    hi = max(poff + QB - qlo for _, qlo, poff in g)
                    for kc, qlo, poff in g:
                        a = poff
                        end = poff + QB - qlo
                        while a < end:
                            b = min((a // 512 + 1) * 512, end)
                            nc.tensor.matmul(
                                ps[:, a:b],
                                KT[half, pr, kc * P:(kc + 1) * P],
                                QT[half, pr, base + qlo + a - poff:
                                   base + qlo + b - poff],
                                start=True, stop=True)
                            a = b
                    pT_t = ptp.tile([P, QB], dt.bfloat16, tag="pT")
                    nc.scalar.activation(pT_t[:, lo:hi], ps[:, lo:hi],
                                         AF.Exp, scale=0.125)
                    for kc, qlo, poff in g:
                        if kc * P >= base:  # diagonal: zero k>q corner
                            nc.gpsimd.affine_select(
                                out=pT_t[:, poff:poff + P],
                                in_=pT_t[:, poff:poff + P],
                                compare_op=ALU.is_ge, fill=0.0,
                                base=0, pattern=[[1, P]], channel_multiplier=-1)
                    for item in pend:
                        emit_pv(*item)
                    pend = [(kc, pT_t, qlo, poff) for kc, qlo, poff in g]
                    if inject is not None and gi in inject:
                        inject[gi]()
                for item in pend:
                    emit_pv(*item)

                # reciprocal of row sums straight from PSUM; evacuate y to
                # SBUF via DVE so the single PSUM accumulator frees quickly;
                # odd heads DMA-shift the bf16 product into partitions 64:128
                pvS = stp.tile([P, QB], dt.float32, tag="pvS")
                rsh = rsp.tile([P, QB], dt.float32, tag="rsh")
                nc.vector.reciprocal(rsh[64:96, :], pv[64:96, :])
                nc.vector.tensor_copy(pvS[0:64, :], pv[0:64, :])
                nc.sync.dma_start(rsh[0:32, :], rsh[64:96, :])
                nc.sync.dma_start(rsh[32:64, :], rsh[64:96, :])
                ysl = slice(qb * QB, qb * QB + QB)
                if parity == 0:
                    nc.vector.tensor_tensor(
                        yT[0:64, pr, ysl], pvS[0:64, :], rsh[0:64, :], ALU.mult)
                else:
                    ytmp = rsp.tile([64, QB], dt.bfloat16, tag="ytmp")
                    nc.vector.tensor_tensor(
                        ytmp[:], pvS[0:64, :], rsh[0:64, :], ALU.mult)
                    nc.sync.dma_start(yT[64:P, pr, ysl], ytmp[:])

            # ---- stage 1: QKV for s-half 0 ----
            for sb in range(2):
                for chp in range(4):
                    qk_unit(0, sb, chp)
            for scp in range(4):
                v_unit(0, scp)

            # ---- stage 2: qb0 attention, half-1 QKV interleaved ----
            h1_units = [lambda sb=sb, chp=chp: qk_unit(1, sb, chp)
                        for sb in range(2) for chp in range(4)]
            h1_units += [lambda scp=scp: v_unit(1, scp) for scp in range(4)]
            for h in range(NHL):
                attn(h, 0)
                if h < 4:
                    h1_units[2 * h]()
                    h1_units[2 * h + 1]()

            # ---- stage 3: qb1 attention, proj half-0 + V half-1 interleaved ----
            vinj = {1 + 2 * i: h1_units[8 + i] for i in range(4)}
            for h in range(NHL):
                attn(h, 1, inject=vinj if h == 0 else None)
                proj_unit(h)

            # ---- stage 4: proj half-1 ----
            for sc in range(8, 16):
                proj_unit(sc)

    nc.compile()
    return nc


def prepare_inputs(x, Wqkv, Wproj):
    """Per-core inputs. Core c: batch c//2, heads (c%2)*8 .. +8."""
    x = np.asarray(x, dtype=np.float32)
    Wqkv = np.asarray(Wqkv, dtype=np.float32)
    Wproj = np.asarray(Wproj, dtype=np.float32)
    Wq = Wqkv[:, :D].reshape(8, P, H, HD)        # [dc, p, head, hd]
    Wk = Wqkv[:, D:2 * D].reshape(8, P, H, HD)
    Wv_ = Wqkv[:, 2 * D:].reshape(8, P, H, HD)
    in_maps = []
    for c in range(8):
        b, g = c // 2, c % 2
        hg = g * NHL
        wqk = np.empty((P, 8, 8, P), dtype=np.float32)
        for ch in range(4):
            wqk[:, :, ch, 0:64] = Wq[:, :, hg + 2 * ch, :].transpose(1, 0, 2)
            wqk[:, :, ch, 64:P] = Wq[:, :, hg + 2 * ch + 1, :].transpose(1, 0, 2)
            wqk[:, :, ch + 4, 0:64] = Wk[:, :, hg + 2 * ch, :].transpose(1, 0, 2)
            wqk[:, :, ch + 4, 64:P] = Wk[:, :, hg + 2 * ch + 1, :].transpose(1, 0, 2)
        wv = Wv_[:, :, hg:hg + NHL, :].transpose(1, 0, 2, 3).reshape(P, 8, 512)
        wpj = np.empty((P, 4, D), dtype=np.float32)
        for pc in range(4):
            wpj[0:64, pc, :] = Wproj[HD * (hg + 2 * pc):HD * (hg + 2 * pc) + HD, :]
            wpj[64:P, pc, :] = Wproj[HD * (hg + 2 * pc + 1):HD * (hg + 2 * pc + 1) + HD, :]
        in_maps.append({
            "xs16": np.ascontiguousarray(x[b]).astype(bfloat16),
            "wqk": wqk.astype(bfloat16),
            "wv": wv.astype(bfloat16),
            "wpj": wpj.astype(bfloat16),
        })
    return in_maps


def combine_outputs(results):
    out = np.empty((B, S, D), dtype=np.float32)
    for b in range(B):
        out[b] = (results[2 * b]["out"].astype(np.float32)
                  + results[2 * b + 1]["out"].astype(np.float32))
    return out


_NC_CACHE = None


def get_nc():
    global _NC_CACHE
    if _NC_CACHE is None:
        _NC_CACHE = build_nc()
    return _NC_CACHE


def kernel(x, Wqkv, Wproj):
    from concourse.bass_utils import run_bass_kernel_spmd
    nc = get_nc()
    in_maps = prepare_inputs(x, Wqkv, Wproj)
    res = run_bass_kernel_spmd(nc, in_maps, core_ids=list(range(8)))
    return combine_outputs(res.results)


if __name__ == "__main__":
    rng = np.random.default_rng(0)
    x = rng.standard_normal((B, S, D), dtype=np.float32)
    Wqkv = (rng.standard_normal((D, 3 * D), dtype=np.float32) / np.sqrt(D)).astype(np.float32)
    Wproj = (rng.standard_normal((D, D), dtype=np.float32) / np.sqrt(D)).astype(np.float32)
    y = kernel(x, Wqkv, Wproj)
    print("ok", y.shape, float(np.abs(y).max()))
